# revision 4
# baseline (speedup 1.0000x reference)
"""ChirpTokenizer Trainium2 kernel.

Math: the reference pipeline (hann window -> per-chirp-rate warp resample
with linear interpolation + jacobian -> rFFT over the warped axis) is
linear in x for each chirp rate d.  It therefore collapses into a single
matmul per d:

    out[b, w, d, f] = sum_k x[b, 512*w + k] * G_d[k, f]

where G_d = diag(hann) @ A_d @ F, A_d is the (K x K_TAU) sparse
interpolation/jacobian matrix (2 nnz per column) and F the rFFT matrix.
Since the input is real, Im(X[0]) = Im(X[1024]) = 0, so the packed real
G_d is (1024 x 2048): [Re f=0..1024 | Im f=1..1023].

G_d depends only on dlnf (16 floats); it is built on the host with a
sparse scatter + FFT (cheap) and shipped to the device.  The device does
pure TensorE work: for each core, 2 chirp rates x (2048 rows x 1024 k x
2048 f) fp32r matmuls, PSUM-accumulated over k.

Sharding: D=16 chirp rates over 8 cores (2 per core); frames replicated.
"""

import os
import numpy as np

K = 1024
HOP = 512
K_TAU = 2048
FK = K_TAU // 2 + 1  # 1025
B = 4
N = 262144
D = 16
NWIN = (N - K) // HOP + 1  # 511
NCORES = 8
DPC = D // NCORES  # 2 chirp rates per core
WPAD = 512  # pad 511 windows -> 512 per batch element
ROWS_PAD = B * WPAD  # 2048

_NC_CACHE = {}


def _warp_grid(dlnf: np.ndarray):
    """Replicate the reference's f32 warp-grid computation bit-for-bit.

    Uses jax on CPU: the grid has a 1/beta cancellation that amplifies
    1-ulp exp/log1p differences into ~1e-3-sample index shifts, so the
    exact XLA-CPU op implementations matter.
    """
    import jax
    import jax.numpy as jnp

    with jax.default_device(jax.local_devices(backend="cpu")[0]):
        beta = 2.0 * jnp.asarray(dlnf, dtype=jnp.float32)
        tau = 2.0 * jnp.arange(K_TAU, dtype=jnp.float32) / K_TAU - 1.0
        small = jnp.abs(beta) < 1e-8
        beta_safe = jnp.where(small, 1e-8, beta)
        e2b = jnp.exp(2.0 * beta_safe)

        t_source = (
            jnp.log1p((tau[None, :] + 1.0) * 0.5 * (e2b[:, None] - 1.0))
            / beta_safe[:, None]
            - 1.0
        )
        t_source = jnp.where(small[:, None], tau[None, :], t_source)

        tau_mid = 2.0 * (K_TAU // 2) / K_TAU - 1.0  # = 0.0
        t_mid = jnp.log1p((tau_mid + 1.0) * 0.5 * (e2b - 1.0)) / beta_safe - 1.0
        t_mid = jnp.where(small, tau_mid, t_mid)

        jac = jnp.exp(-beta_safe[:, None] * (t_source - t_mid[:, None]))
        jac = jnp.where(small[:, None], 1.0, jac)

        idx = (K / 2.0) * (t_source + 1.0)
        idx_lo = jnp.clip(jnp.floor(idx).astype(jnp.int32), 0, K - 2)
        frac = idx - idx_lo.astype(jnp.float32)
        return np.asarray(idx_lo), np.asarray(frac), np.asarray(jac)


def _build_g(dlnf: np.ndarray) -> np.ndarray:
    """(D,) f32 -> (D, 8, 128, 2048) f32: packed DFT-of-resample matrices."""
    f32 = np.float32
    dlnf = dlnf.astype(f32)
    idx_lo, frac, jac = _warp_grid(dlnf)

    w_lo = ((f32(1.0) - frac) * jac).astype(np.float64)  # (D, K_TAU)
    w_hi = (frac * jac).astype(np.float64)

    # A[d, k, t]: sparse scatter (indices are unique — see lo vs lo+1)
    A = np.zeros((D, K, K_TAU), np.float64)
    d_idx = np.repeat(np.arange(D), K_TAU)
    t_idx = np.tile(np.arange(K_TAU), D)
    lo = idx_lo.ravel()
    A[d_idx, lo, t_idx] = w_lo.ravel()
    A[d_idx, lo + 1, t_idx] = w_hi.ravel()

    W = np.fft.rfft(A, axis=-1)  # (D, K, FK) complex128

    n = np.arange(K, dtype=np.float64)
    hann = 0.5 - 0.5 * np.cos(2.0 * np.pi * n / K)

    G = np.empty((D, K, 2048), np.float32)
    G[:, :, :FK] = (W.real * hann[None, :, None]).astype(np.float32)
    G[:, :, FK:] = (W.imag[:, :, 1:1024] * hann[None, :, None]).astype(np.float32)
    return np.ascontiguousarray(G.reshape(D, 8, 128, 2048))


def _build_frames_t(x: np.ndarray) -> np.ndarray:
    """(B, N) f32 -> (8, 128, ROWS_PAD) f32 transposed overlapped frames.

    ft[kc, i, b*512 + w] = x[b, 512*w + 128*kc + i]  (w < 511; w = 511 zero)
    """
    ft = np.zeros((K, ROWS_PAD), np.float32)
    for b in range(B):
        frames = np.lib.stride_tricks.as_strided(
            x[b], shape=(NWIN, K), strides=(HOP * 4, 4)
        )
        ft[:, b * WPAD : b * WPAD + NWIN] = frames.T
    return np.ascontiguousarray(ft.reshape(8, 128, ROWS_PAD))


def _get_nc():
    if "nc" in _NC_CACHE:
        return _NC_CACHE["nc"]
    import concourse.bacc as bacc
    import concourse.mybir as mybir
    from concourse import tile

    nc = bacc.Bacc("TRN2", target_bir_lowering=False, debug=False, num_devices=NCORES)
    ft_d = nc.dram_tensor(
        "ft", [8, 128, ROWS_PAD], mybir.dt.float32r, kind="ExternalInput"
    )
    g_d = nc.dram_tensor(
        "g", [DPC, 8, 128, 2048], mybir.dt.float32r, kind="ExternalInput"
    )
    out_d = nc.dram_tensor(
        "out", [DPC, ROWS_PAD, 2048], mybir.dt.float32, kind="ExternalOutput"
    )

    with tile.TileContext(nc) as tc:
        with (
            tc.tile_pool(name="ftp", bufs=8) as ftp,
            tc.tile_pool(name="gp", bufs=12) as gp,
            tc.tile_pool(name="op", bufs=3) as op,
            tc.tile_pool(name="pp", bufs=4, space="PSUM") as pp,
        ):
            ftt = []
            for kc in range(8):
                t = ftp.tile(
                    [128, ROWS_PAD], mybir.dt.float32r, name=f"ft{kc}", tag="ft"
                )
                nc.sync.dma_start(t[:], ft_d[kc])
                ftt.append(t)
            for d in range(DPC):
                gtt = []
                for kc in range(8):
                    t = gp.tile(
                        [128, 2048], mybir.dt.float32r, name=f"g{d}_{kc}", tag="g"
                    )
                    nc.sync.dma_start(t[:], g_d[d, kc])
                    gtt.append(t)
                for m in range(16):
                    ost = op.tile(
                        [128, 2048], mybir.dt.float32, name=f"o{d}_{m}", tag="o"
                    )
                    for nn in range(4):
                        ps = pp.tile(
                            [128, 512],
                            mybir.dt.float32,
                            name=f"p{d}_{m}_{nn}",
                            tag="p",
                        )
                        for kc in range(8):
                            nc.tensor.matmul(
                                ps[:],
                                ftt[kc][:, 128 * m : 128 * (m + 1)],
                                gtt[kc][:, 512 * nn : 512 * (nn + 1)],
                                start=(kc == 0),
                                stop=(kc == 7),
                            )
                        nc.vector.tensor_copy(ost[:, 512 * nn : 512 * (nn + 1)], ps[:])
                    nc.sync.dma_start(out_d[d, 128 * m : 128 * (m + 1), :], ost[:])
    nc.compile()
    _NC_CACHE["nc"] = nc
    return nc


def _get_runner():
    """Build (once) a sharded jitted callable over the 8 cores.

    Mirrors the multi-core tail of bass2jax.run_bass_via_pjrt, but caches
    the jitted function so repeat kernel() calls don't re-trace/re-compile.
    Returns (fn, in_names, out_names, out_shapes_dtypes).
    """
    if "runner" in _NC_CACHE:
        return _NC_CACHE["runner"]
    import jax
    import concourse.mybir as mybir
    from concourse import bass2jax
    from jax.sharding import Mesh, PartitionSpec
    from jax.experimental.shard_map import shard_map

    nc = _get_nc()
    bass2jax.install_neuronx_cc_hook()

    partition_name = (
        nc.partition_id_tensor.name if nc.partition_id_tensor is not None else None
    )
    in_names = []
    out_names = []
    out_avals = []
    for alloc in nc.m.functions[0].allocations:
        if not isinstance(alloc, mybir.MemoryLocationSet):
            continue
        name = alloc.memorylocations[0].name
        if alloc.kind == "ExternalInput":
            if name != partition_name:
                in_names.append(name)
        elif alloc.kind == "ExternalOutput":
            shape = tuple(alloc.tensor_shape)
            dtype = mybir.dt.np(alloc.dtype)
            out_names.append(name)
            out_avals.append(jax.core.ShapedArray(shape, dtype))
    n_params = len(in_names)
    n_outs = len(out_names)
    all_names = list(in_names) + list(out_names)
    if partition_name is not None:
        all_names.append(partition_name)
    all_names = tuple(all_names)

    def _body(*args):
        operands = list(args)
        if partition_name is not None:
            operands.append(bass2jax.partition_id_tensor())
        outs = bass2jax._bass_exec_p.bind(
            *operands,
            out_avals=tuple(out_avals),
            in_names=all_names,
            out_names=tuple(out_names),
            lowering_input_output_aliases=(),
            sim_require_finite=True,
            sim_require_nnan=True,
            nc=nc,
        )
        return tuple(outs)

    devices = jax.devices()[:NCORES]
    mesh = Mesh(np.asarray(devices), ("core",))
    in_specs = (PartitionSpec("core"),) * (n_params + n_outs)
    out_specs = (PartitionSpec("core"),) * n_outs
    fn = jax.jit(
        shard_map(
            _body, mesh=mesh, in_specs=in_specs, out_specs=out_specs, check_rep=False
        ),
        donate_argnums=tuple(range(n_params, n_params + n_outs)),
        keep_unused=True,
    )
    runner = (fn, in_names, out_names, [(a.shape, a.dtype) for a in out_avals], mesh)
    _NC_CACHE["runner"] = runner
    return runner


def kernel(x: np.ndarray, dlnf: np.ndarray) -> np.ndarray:
    x = np.ascontiguousarray(np.asarray(x, dtype=np.float32))
    dlnf = np.asarray(dlnf, dtype=np.float32)

    G = _build_g(dlnf)  # (D, 8, 128, 2048)
    FT = _build_frames_t(x)  # (8, 128, ROWS_PAD)

    fn, in_names, out_names, out_sd, _mesh = _get_runner()
    per_core = {
        "ft": [FT] * NCORES,
        "g": [G[DPC * c : DPC * (c + 1)] for c in range(NCORES)],
    }
    concat_in = [
        np.ascontiguousarray(np.concatenate(per_core[name], axis=0))
        for name in in_names
    ]
    concat_zeros = [
        np.zeros((NCORES * s[0], *s[1:]), dt) for (s, dt) in out_sd
    ]
    out_arrs = fn(*concat_in, *concat_zeros)
    o_all = np.asarray(out_arrs[out_names.index("out")]).reshape(
        NCORES, DPC, ROWS_PAD, 2048
    )

    out = np.empty((B, NWIN, D, FK), np.complex64)
    for c in range(NCORES):
        for dl in range(DPC):
            d = DPC * c + dl
            arr = o_all[c, dl].reshape(B, WPAD, 2048)[:, :NWIN, :]
            out.real[:, :, d, :] = arr[:, :, :FK]
            out.imag[:, :, d, 1:1024] = arr[:, :, FK:]
            out.imag[:, :, d, 0] = 0.0
            out.imag[:, :, d, 1024] = 0.0
    return out
